# revision 49
# baseline (speedup 1.0000x reference)
from contextlib import ExitStack

import numpy as np

import concourse.bacc as bacc
import concourse.bass as bass
import concourse.mybir as mybir
import concourse.tile as tile
from concourse.bass import ts
from concourse.bass_utils import run_bass_kernel_spmd
from concourse.masks import make_identity

import concourse.dve_ops as dve_ops
from concourse.dve_ops import DveOp
from concourse.dve_spec import (
    Spec, Src0, Src1, C0, C1, C2, One, Zero, sq, minn, maxx, lower,
)
from concourse.dve_uop import DveOpSpec

F32 = mybir.dt.float32
FP16 = mybir.dt.float16
AF = mybir.ActivationFunctionType
AL = mybir.AluOpType

B, C4, T = 16, 128, 512
NCORES = 8
BPC = B // NCORES
NTOK = BPC * C4
W4 = 4 * NTOK
INV_CNT = 1.0 / (C4 * T)
EPS = 1e-5
ISQ2 = float(1.0 / np.sqrt(2.0))

_COMPILED = {}


def _mk_op(name, spec):
    shas = {}
    for ver in ("v3", "v4"):
        try:
            s = DveOpSpec(name=name, opcode=0, uops=lower(spec, ver=ver))
            shas[ver] = s.sha(ver)
        except Exception:
            pass
    return DveOp(name, spec, subdim=False, uops_sha=shas)


def _register_ops():
    have = {op.name for op in dve_ops.OPS}
    out = {}
    w = C0 - Src0
    c = sq(w) * w
    q = (sq(Src0) * w) * C1 + C2
    specs = {
        "KAN_BUMPQ": Spec(
            body=minn(c, q),
            reference=lambda in0, in1, s0, s1, imm2: np.minimum(
                (s0 - in0) ** 3, (s0 - in0) * in0 * in0 * s1 + imm2
            ),
        ),
        "KAN_SILU_U": Spec(
            body=(Src0 * C0 + C1) * Src1,
            reference=lambda in0, in1, s0, s1, imm2: (in0 * s0 + s1) * in1,
        ),
        "KAN_RSQRT_NR": Spec(
            body=((sq(Src1) * Src0) * C0 + C1) * Src1,
            reference=lambda in0, in1, s0, s1, imm2: (
                (in1 * in1 * in0) * s0 + s1
            ) * in1,
        ),
        "KAN_GELU_U": Spec(
            body=((Src0 + C1) * C0) * (Src1 + One) + C2,
            reference=lambda in0, in1, s0, s1, imm2: ((in0 + s1) * s0)
            * (in1 + 1.0) + imm2,
        ),
        "KAN_ABSMIN": Spec(
            body=minn(maxx(Src0 - C0, Zero - (Src0 - C0)), C1),
            reference=lambda in0, in1, s0, s1, imm2: np.minimum(
                np.abs(in0 - s0), s1
            ),
        ),
    }
    for name, spec in specs.items():
        if name in have:
            out[name] = next(op for op in dve_ops.OPS if op.name == name)
            continue
        op = _mk_op(name, spec)
        dve_ops.OPS.append(op)
        dve_ops._SUB_OPCODE_FOR_NAME[name] = (
            dve_ops._CUSTOM_DVE_ROW_BASE + len(dve_ops.OPS) - 1
        )
        dve_ops.CUSTOM_DVE_SPECS[name] = spec
        out[name] = op
    return out


_OPS = _register_ops()
BUMPQ = _OPS["KAN_BUMPQ"]
SILU_U = _OPS["KAN_SILU_U"]
RSQRT_NR = _OPS["KAN_RSQRT_NR"]
GELU_U = _OPS["KAN_GELU_U"]
ABSMIN = _OPS["KAN_ABSMIN"]

JORDER_SPLIT = [0, 5, 6, 7, 8, 1, 2, 3, 4]
JORDER_NAT = list(range(9))


class _KB:
    def __init__(self, nc, tc, ctx):
        self.nc = nc
        self.tc = tc
        p = lambda **kw: ctx.enter_context(tc.tile_pool(**kw))
        self.singles = p(name="singles", bufs=1)
        self.act = p(name="act", bufs=1)
        self.feat = p(name="feat", bufs=2)
        self.sfeat = p(name="sfeat", bufs=2)
        self.sig = p(name="sig", bufs=2)
        self.scr = p(name="scr", bufs=2)
        self.sqscr = p(name="sqscr", bufs=1)
        self.tiny = p(name="tiny", bufs=8)
        self.psum4 = p(name="psum4", bufs=5, space="PSUM")
        self.psum = p(name="psum", bufs=2, space="PSUM")
        self.psum1 = p(name="psum1", bufs=1, space="PSUM")

        self.ident = self.singles.tile([128, 128], F32)
        make_identity(nc, self.ident[:])
        self.ones = self.singles.tile([128, 128], F32)
        nc.gpsimd.memset(self.ones[:], 1.0)
        self.cst = self.singles.tile([128, 16], F32)
        for g in range(8):
            nc.gpsimd.memset(self.cst[:, g : g + 1], -(float(g) + 2.0))
        nc.gpsimd.memset(self.cst[:, 8:9], 5.5)
        nc.gpsimd.memset(self.cst[:, 9:10], -2.2)
        for i in range(4):
            nc.gpsimd.memset(self.cst[0:64, 10 + i : 11 + i], -(float(i) + 2.0))
            nc.gpsimd.memset(self.cst[64:128, 10 + i : 11 + i], -(float(i) + 6.0))
        warm = self.singles.tile([128, 1], F32)
        nc.scalar.activation(warm[:], self.cst[:, 8:9], AF.Sigmoid)

    def silu_dve(self, z_u, Bslot, n, tag):
        nc = self.nc
        sg = self.sig.tile([128, n], FP16, tag=f"sg{n}", name=f"sg_{tag}")
        nc.scalar.activation(sg[:], z_u, AF.Sigmoid, bias=self.cst[:, 9:10],
                             scale=0.4)
        nc.vector._custom_dve(
            SILU_U, out=Bslot, in0=z_u, in1=sg[:], s0=0.4, s1=-2.2
        )

    def feat_big(self, z_u, tag, ndve=0, mid_cb=None):
        nc = self.nc
        zf = z_u[:, :, :]
        Bt = self.feat.tile([128, 9, W4], FP16, tag="featB", name=f"B_{tag}")
        self.silu_dve(zf, Bt[:, 0, :], W4, tag)

        def p1_act(g):
            nc.scalar.activation(
                Bt[:, 1 + g, :], zf, AF.Abs, bias=self.cst[:, g : g + 1],
                scale=1.0,
            )

        def p1_dve(g):
            nc.vector._custom_dve(
                ABSMIN, out=Bt[:, 1 + g, :], in0=zf, s0=float(g) + 2.0,
                s1=2.0,
            )

        def p2(h):
            nc.vector.tensor_scalar(
                out=Bt[:, 1 + 4 * h : 5 + 4 * h, :],
                in0=Bt[:, 1 + 4 * h : 5 + 4 * h, :],
                scalar1=2.0, scalar2=None, op0=AL.min,
            )

        def p3(i):
            nc.vector._custom_dve(
                BUMPQ, out=Bt[:, 1 + 2 * i : 3 + 2 * i, :],
                in0=Bt[:, 1 + 2 * i : 3 + 2 * i, :], s0=2.0, s1=-3.0,
                imm2=4.0,
            )

        if ndve == 4:
            for g in range(4):
                p1_act(g)
            for g in range(4, 8):
                p1_dve(g)
            p3(2); p3(3)
            if mid_cb is not None:
                mid_cb()
            p2(0); p3(0); p3(1)
            return Bt, JORDER_SPLIT
        for g in range(8):
            p1_act(g)
            if g == 3:
                p2(0); p3(0); p3(1)
                if mid_cb is not None:
                    mid_cb()
        p2(1); p3(2); p3(3)
        return Bt, JORDER_NAT

    def feat_small(self, z_u, tag):
        nc = self.nc
        Bt = self.sfeat.tile([128, 5, NTOK], FP16, tag="featS", name=f"B_{tag}")
        self.silu_dve(z_u, Bt[:, 0, :], NTOK, tag)
        for i in range(4):
            nc.scalar.activation(
                Bt[:, 1 + i, :], z_u, AF.Abs,
                bias=self.cst[:, 10 + i : 11 + i], scale=1.0,
            )
        nc.vector.tensor_scalar(
            out=Bt[:, 1:5, :], in0=Bt[:, 1:5, :], scalar1=2.0, scalar2=None,
            op0=AL.min,
        )
        nc.vector._custom_dve(
            BUMPQ, out=Bt[:, 1:5, :], in0=Bt[:, 1:5, :], s0=2.0, s1=-3.0,
            imm2=4.0,
        )
        return Bt

    def kan512(self, Bt, w, jorder, out_cb):
        nc = self.nc
        pms = [
            self.psum4.tile([128, NTOK], F32, tag="pmm", name=f"pmm{m}")
            for m in range(4)
        ]
        for ji, j in enumerate(jorder):
            for k in range(4):
                rhs = Bt[:, j, ts(k, NTOK)]
                for m in range(4):
                    nc.tensor.matmul(
                        pms[m][:], w[:, j, k, m, :], rhs,
                        start=(ji == 0 and k == 0), stop=(ji == 8 and k == 3),
                    )
        for m in range(4):
            out_cb(m, pms[m])

    def kan512_to_64dup(self, Bt, w, jorder, name):
        nc = self.nc
        pm = self.psum1.tile([128, NTOK], F32, tag="pk64", name=name)
        n = 0
        for j in jorder:
            for k in range(4):
                nc.tensor.matmul(
                    pm[:], w[:, j, k, :], Bt[:, j, ts(k, NTOK)],
                    start=(n == 0), stop=(n == 35),
                )
                n += 1
        return pm

    def kan64_to_512(self, Bs, w, out_cb):
        nc = self.nc
        for m in range(4):
            pm = self.psum4.tile([128, NTOK], F32, tag="pmm", name=f"pko{m}")
            for j in range(5):
                nc.tensor.matmul(
                    pm[:], w[:, j, m, :], Bs[:, j, :],
                    start=(j == 0), stop=(j == 4),
                )
            out_cb(m, pm)

    def ln_tiny(self, stats, smap, raw_sq, tag):
        nc = self.nc
        gp = nc.gpsimd
        n = stats.shape[1]
        pstat = self.psum.tile([128, 128], F32, tag="ptr", name=f"pst_{tag}")
        nc.tensor.matmul(pstat[:, :n], self.ones[:], stats[:], start=True,
                         stop=True)
        sG = self.tiny.tile([128, n], F32, name=f"sG_{tag}")
        nc.vector.tensor_scalar(
            out=sG[:], in0=pstat[:, :n], scalar1=INV_CNT, scalar2=None,
            op0=AL.mult,
        )
        mean_u = self.tiny.tile([128, BPC], F32, name=f"mu_{tag}")
        for b in range(BPC):
            idx = [j for j, bb in enumerate(smap) if bb == b]
            dst = mean_u[:, b : b + 1]
            gp.tensor_add(dst, sG[:, idx[0] : idx[0] + 1],
                          sG[:, idx[1] : idx[1] + 1])
            for j in idx[2:]:
                gp.tensor_add(dst, dst, sG[:, j : j + 1])
        e2 = sG[:, n - BPC : n]
        var = self.tiny.tile([128, BPC], F32, name=f"var_{tag}")
        if raw_sq:
            mux = self.tiny.tile([128, BPC], F32, name=f"mux_{tag}")
            gp.tensor_scalar(
                out=mux[:], in0=mean_u[:], scalar1=0.4, scalar2=2.2,
                op0=AL.mult, op1=AL.subtract,
            )
            gp.tensor_mul(var[:], mux[:], mux[:])
            gp.tensor_sub(var[:], e2, var[:])
            eps = EPS
        else:
            gp.tensor_mul(var[:], mean_u[:], mean_u[:])
            gp.tensor_sub(var[:], e2, var[:])
            eps = EPS * 6.25
        aa = self.tiny.tile([128, BPC], F32, name=f"aa_{tag}")
        gp.tensor_scalar_add(aa[:], var[:], eps)
        return aa, mean_u

    def ln_rsqrt(self, aa, mean_u, raw_sq, tag):
        nc = self.nc
        y = self.tiny.tile([128, BPC], F32, name=f"y_{tag}")
        nc.vector.reciprocal(y[:], aa[:])
        nc.vector.tensor_scalar(out=y[:], in0=y[:], scalar1=1.0, scalar2=None,
                                op0=AL.min)
        for _ in range(7):
            nc.vector._custom_dve(
                RSQRT_NR, out=y[:], in0=aa[:], in1=y[:], s0=-0.5, s1=1.5
            )
        if not raw_sq:
            nc.vector.tensor_scalar(out=y[:], in0=y[:], scalar1=2.5,
                                    scalar2=None, op0=AL.mult)
        d = self.tiny.tile([128, BPC], F32, name=f"d_{tag}")
        nc.vector.tensor_mul(d[:], y[:], mean_u[:])
        nc.vector.tensor_scalar(out=d[:], in0=d[:], scalar1=5.5, scalar2=None,
                                op0=AL.subtract)
        return y, d

    def ln_apply(self, u, a, d, tag, lnw=None, lnb=None, eng=None):
        nc = self.nc
        eng = eng or nc.gpsimd
        z = self.act.tile([128, 4, NTOK], F32, tag=tag)
        for b in range(BPC):
            eng.tensor_scalar(
                out=z[:, :, ts(b, C4)], in0=u[:, :, ts(b, C4)],
                scalar1=a[:, b : b + 1], scalar2=d[:, b : b + 1],
                op0=AL.mult, op1=AL.subtract,
            )
        if lnw is not None:
            eng.tensor_mul(z[:], z[:], lnw[:])
        if lnb is not None:
            eng.tensor_add(z[:], z[:], lnb[:])
        return z

    def gcn(self, tm16, wg, bg, yname, u_out):
        nc = self.nc
        gp = nc.gpsimd
        y = self.act.tile([128, 4, NTOK], F32, tag=yname)
        for m in range(4):
            pm = self.psum4.tile([128, NTOK], F32, tag="pmm", name=f"pg{m}")
            for k in range(4):
                nc.tensor.matmul(
                    pm[:], wg[:, k, m, :], tm16[:, k, :],
                    start=(k == 0), stop=(k == 3),
                )
            e = self.scr.tile([128, NTOK], F32, tag="erf", name=f"e{m}")
            nc.scalar.activation(
                e[:], pm[:], AF.Erf, bias=bg[:, m, 1:2], scale=ISQ2
            )
            if u_out:
                nc.vector._custom_dve(
                    GELU_U, out=y[:, m, :], in0=pm[:], in1=e[:],
                    s0=1.25, s1=bg[:, m, 0:1], imm2=5.5,
                )
            else:
                hb = self.scr.tile([128, NTOK], F32, tag="hb", name=f"hb{m}")
                nc.scalar.activation(
                    hb[:], pm[:], AF.Identity, bias=bg[:, m, 2:3], scale=0.5
                )
                t1 = self.scr.tile([128, NTOK], F32, tag="erf", name=f"t1{m}")
                gp.tensor_scalar_add(t1[:], e[:], 1.0)
                gp.tensor_mul(y[:, m, :], hb[:], t1[:])
        return y


def _emit(nc, ln_flags):
    use_lnw1, use_lnb1, use_lnw2, use_lnb2 = ln_flags
    dram = {}

    def din(name, shape, dt=FP16):
        dram[name] = nc.dram_tensor(name, shape, dt, kind="ExternalInput").ap()
        return dram[name]

    x_d = din("x_sh", (BPC, C4, T), F32)
    w_k1 = din("w_k1", (128, 9, 4, 4, 128))
    w_tm1k1 = din("w_tm1k1", (128, 9, 4, 128))
    w_tm1k2 = din("w_tm1k2", (128, 5, 4, 128))
    w_g1 = din("w_g1", (128, 4, 4, 128))
    b_g1 = din("b_g1", (128, 4, 3), F32)
    w_tm2k1 = din("w_tm2k1", (128, 9, 4, 128))
    w_tm2k2 = din("w_tm2k2", (128, 5, 4, 128))
    w_g2 = din("w_g2", (128, 4, 4, 128))
    b_g2 = din("b_g2", (128, 4, 3), F32)
    w_k2 = din("w_k2", (128, 9, 4, 4, 128))
    ln1w_d = din("ln1w", (128, 4, NTOK), F32) if use_lnw1 else None
    ln1b_d = din("ln1b", (128, 4, NTOK), F32) if use_lnb1 else None
    ln2w_d = din("ln2w", (128, 4, NTOK), F32) if use_lnw2 else None
    ln2b_d = din("ln2b", (128, 4, NTOK), F32) if use_lnb2 else None
    out_d = nc.dram_tensor("out_sh", (BPC, C4, T), F32, kind="ExternalOutput").ap()

    with tile.TileContext(nc) as tc, ExitStack() as ctx:
        kb = _KB(nc, tc, ctx)
        wpool = ctx.enter_context(tc.tile_pool(name="weights", bufs=1))
        sync = nc.sync

        def wload(ap, shape, tag, dt=FP16, nchunk=1):
            t = wpool.tile(list(shape), dt, tag=tag)
            if nchunk == 1:
                sync.dma_start(t[:], ap)
            else:
                step = shape[1] // nchunk
                for c in range(nchunk):
                    sl = slice(c * step, (c + 1) * step)
                    sync.dma_start(t[:, sl], ap[:, sl])
            return t

        xN = kb.act.tile([C4, BPC, T], F32, tag="nat")
        x_r = x_d.rearrange("b p t -> p b t")
        for b in range(BPC):
            sync.dma_start(xN[:, b, :], x_r[:, b, :])
        W_k1 = wload(w_k1, (128, 9, 4, 4, 128), "wk1", nchunk=3)
        W_tm1k1 = wload(w_tm1k1, (128, 9, 4, 128), "wtm1k1")
        W_tm1k2 = wload(w_tm1k2, (128, 5, 4, 128), "wtm1k2")
        W_g1 = wload(w_g1, (128, 4, 4, 128), "wg1")
        B_g1 = wload(b_g1, (128, 4, 3), "bg1", F32)
        W_k2 = wload(w_k2, (128, 9, 4, 4, 128), "wk2", nchunk=3)
        W_tm2k1 = wload(w_tm2k1, (128, 9, 4, 128), "wtm2k1")
        W_tm2k2 = wload(w_tm2k2, (128, 5, 4, 128), "wtm2k2")
        W_g2 = wload(w_g2, (128, 4, 4, 128), "wg2")
        B_g2 = wload(b_g2, (128, 4, 3), "bg2", F32)
        LN1W = wload(ln1w_d, (128, 4, NTOK), "ln1w", F32) if use_lnw1 else None
        LN1B = wload(ln1b_d, (128, 4, NTOK), "ln1b", F32) if use_lnb1 else None
        LN2W = wload(ln2w_d, (128, 4, NTOK), "ln2w", F32) if use_lnw2 else None
        LN2B = wload(ln2b_d, (128, 4, NTOK), "ln2b", F32) if use_lnb2 else None

        stats1 = kb.tiny.tile([128, 10], F32, name="stats1")
        u_x = kb.act.tile([128, 4, NTOK], F32, tag="ux")
        for b in range(BPC):
            for k in range(4):
                pt = kb.psum.tile([128, 128], F32, tag="ptr")
                nc.tensor.transpose(pt[:], xN[:, b, ts(k, 128)], kb.ident[:])
                nc.scalar.activation(
                    u_x[:, k, ts(b, 128)], pt[:], AF.Identity,
                    bias=kb.cst[:, 8:9], scale=2.5,
                    accum_out=stats1[:, 4 * b + k : 4 * b + k + 1],
                )
            sqx = kb.sqscr.tile([128, T], F32, tag="sqx", name=f"sqx{b}")
            nc.scalar.activation(
                sqx[:], xN[:, b, :], AF.Square,
                accum_out=stats1[:, 8 + b : 9 + b],
            )
        aa1, mu1 = kb.ln_tiny(stats1, [0, 0, 0, 0, 1, 1, 1, 1], True, "ln1")

        _h = {}

        def _mid1():
            a1, d1 = kb.ln_rsqrt(aa1, mu1, True, "ln1")
            _h["z1"] = kb.ln_apply(u_x, a1, d1, "z1o", LN1W, LN1B)

        BX, joX = kb.feat_big(u_x, "x", ndve=4, mid_cb=_mid1)
        z1 = _h["z1"]
        BA, joA = kb.feat_big(z1, "a", ndve=0)

        stats2 = kb.tiny.tile([128, 10], F32, name="stats2")
        cm_u = kb.act.tile([128, 4, NTOK], F32, tag="cmy")

        def cm_cb(m, pm):
            for b in range(BPC):
                nc.scalar.activation(
                    cm_u[:, m, ts(b, 128)], pm[:, ts(b, 128)], AF.Identity,
                    bias=kb.cst[:, 8:9], scale=2.5,
                    accum_out=stats2[:, 4 * b + m : 4 * b + m + 1],
                )

        kb.kan512(BX, W_k1, joX, cm_cb)
        for b in range(BPC):
            sqc = kb.sqscr.tile([128, 4, 128], F32, tag="sqc", name=f"sqc{b}")
            nc.scalar.activation(
                sqc[:], cm_u[:, :, ts(b, C4)], AF.Square,
                accum_out=stats2[:, 8 + b : 9 + b],
            )
        aa2, mu2 = kb.ln_tiny(stats2, [0, 0, 0, 0, 1, 1, 1, 1], False, "ln2")
        a2, d2 = kb.ln_rsqrt(aa2, mu2, False, "ln2")
        z3 = kb.ln_apply(cm_u, a2, d2, "z3u", LN2W, LN2B, eng=nc.vector)

        BC, joC = kb.feat_big(z3, "c", ndve=4)

        p1 = kb.kan512_to_64dup(BA, W_tm1k1, joA, "p1")
        z2 = kb.act.tile([128, NTOK], F32, tag="z2u", name="z2u")
        nc.vector.tensor_scalar(
            out=z2[:], in0=p1[:], scalar1=2.5, scalar2=5.5,
            op0=AL.mult, op1=AL.add,
        )

        p2 = kb.kan512_to_64dup(BC, W_tm2k1, joC, "p2")
        z4 = kb.act.tile([128, NTOK], F32, tag="z4u", name="z4u")
        nc.vector.tensor_scalar(
            out=z4[:], in0=p2[:], scalar1=2.5, scalar2=5.5,
            op0=AL.mult, op1=AL.add,
        )
        BD = kb.feat_small(z4, "d")
        BB = kb.feat_small(z2, "b")

        tm2 = kb.sig.tile([128, 4, NTOK], FP16, tag="tmu", name="tm2")
        kb.kan64_to_512(
            BD, W_tm2k2,
            lambda m, pm: nc.vector.tensor_add(tm2[:, m, :], cm_u[:, m, :],
                                               pm[:]),
        )
        y2u = kb.gcn(tm2, W_g2, B_g2, "y2u", u_out=True)

        tm1 = kb.sig.tile([128, 4, NTOK], FP16, tag="tmu", name="tm1")
        kb.kan64_to_512(
            BB, W_tm1k2,
            lambda m, pm: nc.vector.tensor_add(tm1[:, m, :], u_x[:, m, :],
                                               pm[:]),
        )
        y1 = kb.gcn(tm1, W_g1, B_g1, "cmy", u_out=False)

        BY, joY = kb.feat_big(y2u, "y", ndve=4)
        outT = kb.act.tile([128, 4, NTOK], F32, tag="z1o", name="outT")
        kb.kan512(
            BY, W_k2, joY,
            lambda m, pm: nc.vector.tensor_add(outT[:, m, :], y1[:, m, :],
                                               pm[:]),
        )

        outN = kb.act.tile([C4, BPC, T], F32, tag="nat", name="outN")
        out_r = out_d.rearrange("b p t -> p b t")
        for m in range(4):
            for b in range(BPC):
                pt = kb.psum.tile([128, 128], F32, tag="ptr")
                nc.tensor.transpose(pt[:], outT[:, m, ts(b, 128)], kb.ident[:])
                nc.scalar.copy(outN[:, b, ts(m, 128)], pt[:])
        for b in range(BPC):
            sync.dma_start(out_r[:, b, :], outN[:, b, :])

    return dram


def _build(ln_flags):
    key = ln_flags
    if key in _COMPILED:
        return _COMPILED[key]
    nc = bacc.Bacc("TRN2", target_bir_lowering=False, debug=False)
    _emit(nc, ln_flags)
    nc.compile()
    _COMPILED[key] = nc
    return nc


def _prep_kan512(base_w, spline_w):
    w = np.empty((128, 9, 4, 4, 128), np.float32)
    for k in range(4):
        for m in range(4):
            w[:, 0, k, m, :] = base_w[m * 128 : (m + 1) * 128,
                                      k * 128 : (k + 1) * 128].T
            for g in range(8):
                w[:, 1 + g, k, m, :] = (
                    spline_w[m * 128 : (m + 1) * 128,
                             k * 128 : (k + 1) * 128, g].T / 6.0
                )
    return np.ascontiguousarray(w.astype(np.float16))


def _prep_kan512_to_64dup(base_w, spline_w):
    w = np.empty((128, 9, 4, 128), np.float32)
    for k in range(4):
        blk = base_w[:, k * 128 : (k + 1) * 128].T
        w[:, 0, k, 0:64] = blk
        w[:, 0, k, 64:128] = blk
        for g in range(8):
            sblk = spline_w[:, k * 128 : (k + 1) * 128, g].T / 6.0
            w[:, 1 + g, k, 0:64] = sblk
            w[:, 1 + g, k, 64:128] = sblk
    return np.ascontiguousarray(w.astype(np.float16))


def _prep_kan64_to_512(base_w, spline_w, scale):
    w = np.zeros((128, 5, 4, 128), np.float32)
    for m in range(4):
        w[0:64, 0, m, :] = base_w[m * 128 : (m + 1) * 128, :].T * scale
        for i in range(4):
            w[0:64, 1 + i, m, :] = (
                spline_w[m * 128 : (m + 1) * 128, :, i].T * (scale / 6.0)
            )
            w[64:128, 1 + i, m, :] = (
                spline_w[m * 128 : (m + 1) * 128, :, i + 4].T * (scale / 6.0)
            )
    return np.ascontiguousarray(w.astype(np.float16))


def _prep_gcn(gw, gb):
    Wf = gw[:, :512] + gw[:, 512:1024] + gw[:, 1024:]
    w = np.empty((128, 4, 4, 128), np.float32)
    for k in range(4):
        for m in range(4):
            w[:, k, m, :] = (
                Wf[m * 128 : (m + 1) * 128, k * 128 : (k + 1) * 128].T / 2.5
            )
    badj = gb - 2.2 * Wf.sum(axis=1)
    b = np.empty((128, 4, 3), np.float32)
    b[:, :, 0] = badj.reshape(4, 128).T
    b[:, :, 1] = b[:, :, 0] * ISQ2
    b[:, :, 2] = b[:, :, 0] * 0.5
    return np.ascontiguousarray(w.astype(np.float16)), np.ascontiguousarray(b)


def _ln_planes(w, b):
    W = np.empty((128, 4, NTOK), np.float32)
    Bp = np.empty((128, 4, NTOK), np.float32)
    bb = 2.5 * b + 5.5 * (1.0 - w)
    for k in range(4):
        for bt in range(BPC):
            W[:, k, bt * C4 : (bt + 1) * C4] = w[k * 128 : (k + 1) * 128, :]
            Bp[:, k, bt * C4 : (bt + 1) * C4] = bb[k * 128 : (k + 1) * 128, :]
    return np.ascontiguousarray(W), np.ascontiguousarray(Bp)


def kernel(**inputs):
    i = {k: np.asarray(v) for k, v in inputs.items()}
    use_lnw1 = not np.all(i["tm1_ln_w"] == 1.0)
    use_lnb1 = not np.all(i["tm1_ln_b"] == 0.0)
    use_lnw2 = not np.all(i["tm_ln_w"] == 1.0)
    use_lnb2 = not np.all(i["tm_ln_b"] == 0.0)
    ln_flags = (use_lnw1, use_lnb1, use_lnw2, use_lnb2)
    nc = _build(ln_flags)

    w_g1, b_g1 = _prep_gcn(i["g1_w"], i["g1_b"])
    w_g2, b_g2 = _prep_gcn(i["g2_w"], i["g2_b"])
    shared = dict(
        w_k1=_prep_kan512(i["k1_base"], i["k1_spline"]),
        w_tm1k1=_prep_kan512_to_64dup(i["tm1_k1_base"], i["tm1_k1_spline"]),
        w_tm1k2=_prep_kan64_to_512(i["tm1_k2_base"], i["tm1_k2_spline"], 2.5),
        w_g1=w_g1, b_g1=b_g1,
        w_tm2k1=_prep_kan512_to_64dup(i["tm_k1_base"], i["tm_k1_spline"]),
        w_tm2k2=_prep_kan64_to_512(i["tm_k2_base"], i["tm_k2_spline"], 2.5),
        w_g2=w_g2, b_g2=b_g2,
        w_k2=_prep_kan512(i["k2_base"], i["k2_spline"]),
    )
    if use_lnw1 or use_lnb1:
        W, Bp = _ln_planes(i["tm1_ln_w"], i["tm1_ln_b"])
        if use_lnw1:
            shared["ln1w"] = W
        if use_lnb1:
            shared["ln1b"] = Bp
    if use_lnw2 or use_lnb2:
        W, Bp = _ln_planes(i["tm_ln_w"], i["tm_ln_b"])
        if use_lnw2:
            shared["ln2w"] = W
        if use_lnb2:
            shared["ln2b"] = Bp
    x = np.ascontiguousarray(i["x"], np.float32)
    in_maps = [
        {"x_sh": x[c * BPC : (c + 1) * BPC], **shared} for c in range(NCORES)
    ]
    res = run_bass_kernel_spmd(nc, in_maps, core_ids=list(range(NCORES)))
    out = np.empty((B, C4, T), np.float32)
    for c in range(NCORES):
        out[c * BPC : (c + 1) * BPC] = res.results[c]["out_sh"]
    return out


# revision 50
# speedup vs baseline: 1.0060x; 1.0060x over previous
from contextlib import ExitStack

import numpy as np

import concourse.bacc as bacc
import concourse.bass as bass
import concourse.mybir as mybir
import concourse.tile as tile
from concourse.bass import ts
from concourse.bass_utils import run_bass_kernel_spmd
from concourse.masks import make_identity

import concourse.dve_ops as dve_ops
from concourse.dve_ops import DveOp
from concourse.dve_spec import (
    Spec, Src0, Src1, C0, C1, C2, One, Zero, sq, minn, maxx, lower,
)
from concourse.dve_uop import DveOpSpec

F32 = mybir.dt.float32
FP16 = mybir.dt.float16
AF = mybir.ActivationFunctionType
AL = mybir.AluOpType

B, C4, T = 16, 128, 512
NCORES = 8
BPC = B // NCORES
NTOK = BPC * C4
W4 = 4 * NTOK
INV_CNT = 1.0 / (C4 * T)
EPS = 1e-5
ISQ2 = float(1.0 / np.sqrt(2.0))

_COMPILED = {}


def _mk_op(name, spec):
    shas = {}
    for ver in ("v3", "v4"):
        try:
            s = DveOpSpec(name=name, opcode=0, uops=lower(spec, ver=ver))
            shas[ver] = s.sha(ver)
        except Exception:
            pass
    return DveOp(name, spec, subdim=False, uops_sha=shas)


def _register_ops():
    have = {op.name for op in dve_ops.OPS}
    out = {}
    w = C0 - Src0
    c = sq(w) * w
    q = (sq(Src0) * w) * C1 + C2
    specs = {
        "KAN_BUMPQ": Spec(
            body=minn(c, q),
            reference=lambda in0, in1, s0, s1, imm2: np.minimum(
                (s0 - in0) ** 3, (s0 - in0) * in0 * in0 * s1 + imm2
            ),
        ),
        "KAN_SILU_U": Spec(
            body=(Src0 * C0 + C1) * Src1,
            reference=lambda in0, in1, s0, s1, imm2: (in0 * s0 + s1) * in1,
        ),
        "KAN_RSQRT_NR": Spec(
            body=((sq(Src1) * Src0) * C0 + C1) * Src1,
            reference=lambda in0, in1, s0, s1, imm2: (
                (in1 * in1 * in0) * s0 + s1
            ) * in1,
        ),
        "KAN_GELU_U": Spec(
            body=((Src0 + C1) * C0) * (Src1 + One) + C2,
            reference=lambda in0, in1, s0, s1, imm2: ((in0 + s1) * s0)
            * (in1 + 1.0) + imm2,
        ),
        "KAN_ABSMIN": Spec(
            body=minn(maxx(Src0 - C0, Zero - (Src0 - C0)), C1),
            reference=lambda in0, in1, s0, s1, imm2: np.minimum(
                np.abs(in0 - s0), s1
            ),
        ),
    }
    for name, spec in specs.items():
        if name in have:
            out[name] = next(op for op in dve_ops.OPS if op.name == name)
            continue
        op = _mk_op(name, spec)
        dve_ops.OPS.append(op)
        dve_ops._SUB_OPCODE_FOR_NAME[name] = (
            dve_ops._CUSTOM_DVE_ROW_BASE + len(dve_ops.OPS) - 1
        )
        dve_ops.CUSTOM_DVE_SPECS[name] = spec
        out[name] = op
    return out


_OPS = _register_ops()
BUMPQ = _OPS["KAN_BUMPQ"]
SILU_U = _OPS["KAN_SILU_U"]
RSQRT_NR = _OPS["KAN_RSQRT_NR"]
GELU_U = _OPS["KAN_GELU_U"]
ABSMIN = _OPS["KAN_ABSMIN"]

JORDER_SPLIT = [0, 5, 6, 7, 8, 1, 2, 3, 4]
JORDER_NAT = list(range(9))


class _KB:
    def __init__(self, nc, tc, ctx):
        self.nc = nc
        self.tc = tc
        p = lambda **kw: ctx.enter_context(tc.tile_pool(**kw))
        self.singles = p(name="singles", bufs=1)
        self.act = p(name="act", bufs=1)
        self.feat = p(name="feat", bufs=2)
        self.sfeat = p(name="sfeat", bufs=2)
        self.sig = p(name="sig", bufs=2)
        self.scr = p(name="scr", bufs=2)
        self.sqscr = p(name="sqscr", bufs=1)
        self.tiny = p(name="tiny", bufs=8)
        self.psum4 = p(name="psum4", bufs=5, space="PSUM")
        self.psum = p(name="psum", bufs=2, space="PSUM")
        self.psum1 = p(name="psum1", bufs=1, space="PSUM")

        self.ident = self.singles.tile([128, 128], F32)
        make_identity(nc, self.ident[:])
        self.ones = self.singles.tile([128, 128], F32)
        nc.gpsimd.memset(self.ones[:], 1.0)
        self.cst = self.singles.tile([128, 16], F32)
        for g in range(8):
            nc.gpsimd.memset(self.cst[:, g : g + 1], -(float(g) + 2.0))
        nc.gpsimd.memset(self.cst[:, 8:9], 5.5)
        nc.gpsimd.memset(self.cst[:, 9:10], -2.2)
        for i in range(4):
            nc.gpsimd.memset(self.cst[0:64, 10 + i : 11 + i], -(float(i) + 2.0))
            nc.gpsimd.memset(self.cst[64:128, 10 + i : 11 + i], -(float(i) + 6.0))
        warm = self.singles.tile([128, 1], F32)
        nc.scalar.activation(warm[:], self.cst[:, 8:9], AF.Sigmoid)

    def silu_dve(self, z_u, Bslot, n, tag):
        nc = self.nc
        sg = self.sig.tile([128, n], FP16, tag=f"sg{n}", name=f"sg_{tag}")
        nc.scalar.activation(sg[:], z_u, AF.Sigmoid, bias=self.cst[:, 9:10],
                             scale=0.4)
        nc.vector._custom_dve(
            SILU_U, out=Bslot, in0=z_u, in1=sg[:], s0=0.4, s1=-2.2
        )

    def feat_big(self, z_u, tag, ndve=0, mid_cb=None):
        nc = self.nc
        zf = z_u[:, :, :]
        Bt = self.feat.tile([128, 9, W4], FP16, tag="featB", name=f"B_{tag}")
        self.silu_dve(zf, Bt[:, 0, :], W4, tag)

        def p1_act(g):
            nc.scalar.activation(
                Bt[:, 1 + g, :], zf, AF.Abs, bias=self.cst[:, g : g + 1],
                scale=1.0,
            )

        def p1_dve(g):
            nc.vector._custom_dve(
                ABSMIN, out=Bt[:, 1 + g, :], in0=zf, s0=float(g) + 2.0,
                s1=2.0,
            )

        def p2(h):
            nc.vector.tensor_scalar(
                out=Bt[:, 1 + 4 * h : 5 + 4 * h, :],
                in0=Bt[:, 1 + 4 * h : 5 + 4 * h, :],
                scalar1=2.0, scalar2=None, op0=AL.min,
            )

        def p3(i):
            nc.vector._custom_dve(
                BUMPQ, out=Bt[:, 1 + 2 * i : 3 + 2 * i, :],
                in0=Bt[:, 1 + 2 * i : 3 + 2 * i, :], s0=2.0, s1=-3.0,
                imm2=4.0,
            )

        if ndve == 4:
            for g in range(4):
                p1_act(g)
            for g in range(4, 8):
                p1_dve(g)
            p3(2); p3(3)
            if mid_cb is not None:
                mid_cb()
            p2(0); p3(0); p3(1)
            return Bt, JORDER_SPLIT
        for g in range(8):
            p1_act(g)
            if g == 3:
                p2(0); p3(0); p3(1)
                if mid_cb is not None:
                    mid_cb()
        p2(1); p3(2); p3(3)
        return Bt, JORDER_NAT

    def feat_small(self, z_u, tag):
        nc = self.nc
        Bt = self.sfeat.tile([128, 5, NTOK], FP16, tag="featS", name=f"B_{tag}")
        self.silu_dve(z_u, Bt[:, 0, :], NTOK, tag)
        for i in range(4):
            nc.scalar.activation(
                Bt[:, 1 + i, :], z_u, AF.Abs,
                bias=self.cst[:, 10 + i : 11 + i], scale=1.0,
            )
        nc.vector.tensor_scalar(
            out=Bt[:, 1:5, :], in0=Bt[:, 1:5, :], scalar1=2.0, scalar2=None,
            op0=AL.min,
        )
        nc.vector._custom_dve(
            BUMPQ, out=Bt[:, 1:5, :], in0=Bt[:, 1:5, :], s0=2.0, s1=-3.0,
            imm2=4.0,
        )
        return Bt

    def kan512(self, Bt, w, jorder, out_cb):
        nc = self.nc
        pms = [
            self.psum4.tile([128, NTOK], F32, tag="pmm", name=f"pmm{m}")
            for m in range(4)
        ]
        for ji, j in enumerate(jorder):
            for k in range(4):
                rhs = Bt[:, j, ts(k, NTOK)]
                for m in range(4):
                    nc.tensor.matmul(
                        pms[m][:], w[:, j, k, m, :], rhs,
                        start=(ji == 0 and k == 0), stop=(ji == 8 and k == 3),
                    )
        for m in range(4):
            out_cb(m, pms[m])

    def kan512_to_64dup(self, Bt, w, jorder, name):
        nc = self.nc
        pm = self.psum1.tile([128, NTOK], F32, tag="pk64", name=name)
        n = 0
        for j in jorder:
            for k in range(4):
                nc.tensor.matmul(
                    pm[:], w[:, j, k, :], Bt[:, j, ts(k, NTOK)],
                    start=(n == 0), stop=(n == 35),
                )
                n += 1
        return pm

    def kan64_to_512(self, Bs, w, out_cb):
        nc = self.nc
        for m in range(4):
            pm = self.psum4.tile([128, NTOK], F32, tag="pmm", name=f"pko{m}")
            for j in range(5):
                nc.tensor.matmul(
                    pm[:], w[:, j, m, :], Bs[:, j, :],
                    start=(j == 0), stop=(j == 4),
                )
            out_cb(m, pm)

    def ln_tiny(self, stats, smap, raw_sq, tag):
        nc = self.nc
        gp = nc.gpsimd
        n = stats.shape[1]
        pstat = self.psum.tile([128, 128], F32, tag="ptr", name=f"pst_{tag}")
        nc.tensor.matmul(pstat[:, :n], self.ones[:], stats[:], start=True,
                         stop=True)
        sG = self.tiny.tile([128, n], F32, name=f"sG_{tag}")
        nc.vector.tensor_scalar(
            out=sG[:], in0=pstat[:, :n], scalar1=INV_CNT, scalar2=None,
            op0=AL.mult,
        )
        mean_u = self.tiny.tile([128, BPC], F32, name=f"mu_{tag}")
        for b in range(BPC):
            idx = [j for j, bb in enumerate(smap) if bb == b]
            dst = mean_u[:, b : b + 1]
            gp.tensor_add(dst, sG[:, idx[0] : idx[0] + 1],
                          sG[:, idx[1] : idx[1] + 1])
            for j in idx[2:]:
                gp.tensor_add(dst, dst, sG[:, j : j + 1])
        e2 = sG[:, n - BPC : n]
        var = self.tiny.tile([128, BPC], F32, name=f"var_{tag}")
        if raw_sq:
            mux = self.tiny.tile([128, BPC], F32, name=f"mux_{tag}")
            gp.tensor_scalar(
                out=mux[:], in0=mean_u[:], scalar1=0.4, scalar2=2.2,
                op0=AL.mult, op1=AL.subtract,
            )
            gp.tensor_mul(var[:], mux[:], mux[:])
            gp.tensor_sub(var[:], e2, var[:])
            eps = EPS
        else:
            gp.tensor_mul(var[:], mean_u[:], mean_u[:])
            gp.tensor_sub(var[:], e2, var[:])
            eps = EPS * 6.25
        aa = self.tiny.tile([128, BPC], F32, name=f"aa_{tag}")
        gp.tensor_scalar_add(aa[:], var[:], eps)
        return aa, mean_u

    def ln_rsqrt(self, aa, mean_u, raw_sq, tag):
        nc = self.nc
        y = self.tiny.tile([128, BPC], F32, name=f"y_{tag}")
        nc.vector.reciprocal(y[:], aa[:])
        nc.vector.tensor_scalar(out=y[:], in0=y[:], scalar1=1.0, scalar2=None,
                                op0=AL.min)
        for _ in range(3 if raw_sq else 7):
            nc.vector._custom_dve(
                RSQRT_NR, out=y[:], in0=aa[:], in1=y[:], s0=-0.5, s1=1.5
            )
        if not raw_sq:
            nc.vector.tensor_scalar(out=y[:], in0=y[:], scalar1=2.5,
                                    scalar2=None, op0=AL.mult)
        d = self.tiny.tile([128, BPC], F32, name=f"d_{tag}")
        nc.vector.tensor_mul(d[:], y[:], mean_u[:])
        nc.vector.tensor_scalar(out=d[:], in0=d[:], scalar1=5.5, scalar2=None,
                                op0=AL.subtract)
        return y, d

    def ln_apply(self, u, a, d, tag, lnw=None, lnb=None, eng=None):
        nc = self.nc
        eng = eng or nc.gpsimd
        z = self.act.tile([128, 4, NTOK], F32, tag=tag)
        for b in range(BPC):
            eng.tensor_scalar(
                out=z[:, :, ts(b, C4)], in0=u[:, :, ts(b, C4)],
                scalar1=a[:, b : b + 1], scalar2=d[:, b : b + 1],
                op0=AL.mult, op1=AL.subtract,
            )
        if lnw is not None:
            eng.tensor_mul(z[:], z[:], lnw[:])
        if lnb is not None:
            eng.tensor_add(z[:], z[:], lnb[:])
        return z

    def gcn(self, tm16, wg, bg, yname, u_out):
        nc = self.nc
        gp = nc.gpsimd
        y = self.act.tile([128, 4, NTOK], F32, tag=yname)
        for m in range(4):
            pm = self.psum4.tile([128, NTOK], F32, tag="pmm", name=f"pg{m}")
            for k in range(4):
                nc.tensor.matmul(
                    pm[:], wg[:, k, m, :], tm16[:, k, :],
                    start=(k == 0), stop=(k == 3),
                )
            e = self.scr.tile([128, NTOK], F32, tag="erf", name=f"e{m}")
            nc.scalar.activation(
                e[:], pm[:], AF.Erf, bias=bg[:, m, 1:2], scale=ISQ2
            )
            if u_out:
                nc.vector._custom_dve(
                    GELU_U, out=y[:, m, :], in0=pm[:], in1=e[:],
                    s0=1.25, s1=bg[:, m, 0:1], imm2=5.5,
                )
            else:
                hb = self.scr.tile([128, NTOK], F32, tag="hb", name=f"hb{m}")
                nc.scalar.activation(
                    hb[:], pm[:], AF.Identity, bias=bg[:, m, 2:3], scale=0.5
                )
                t1 = self.scr.tile([128, NTOK], F32, tag="erf", name=f"t1{m}")
                gp.tensor_scalar_add(t1[:], e[:], 1.0)
                gp.tensor_mul(y[:, m, :], hb[:], t1[:])
        return y


def _emit(nc, ln_flags):
    use_lnw1, use_lnb1, use_lnw2, use_lnb2 = ln_flags
    dram = {}

    def din(name, shape, dt=FP16):
        dram[name] = nc.dram_tensor(name, shape, dt, kind="ExternalInput").ap()
        return dram[name]

    x_d = din("x_sh", (BPC, C4, T), F32)
    w_k1 = din("w_k1", (128, 9, 4, 4, 128))
    w_tm1k1 = din("w_tm1k1", (128, 9, 4, 128))
    w_tm1k2 = din("w_tm1k2", (128, 5, 4, 128))
    w_g1 = din("w_g1", (128, 4, 4, 128))
    b_g1 = din("b_g1", (128, 4, 3), F32)
    w_tm2k1 = din("w_tm2k1", (128, 9, 4, 128))
    w_tm2k2 = din("w_tm2k2", (128, 5, 4, 128))
    w_g2 = din("w_g2", (128, 4, 4, 128))
    b_g2 = din("b_g2", (128, 4, 3), F32)
    w_k2 = din("w_k2", (128, 9, 4, 4, 128))
    ln1w_d = din("ln1w", (128, 4, NTOK), F32) if use_lnw1 else None
    ln1b_d = din("ln1b", (128, 4, NTOK), F32) if use_lnb1 else None
    ln2w_d = din("ln2w", (128, 4, NTOK), F32) if use_lnw2 else None
    ln2b_d = din("ln2b", (128, 4, NTOK), F32) if use_lnb2 else None
    out_d = nc.dram_tensor("out_sh", (BPC, C4, T), F32, kind="ExternalOutput").ap()

    with tile.TileContext(nc) as tc, ExitStack() as ctx:
        kb = _KB(nc, tc, ctx)
        wpool = ctx.enter_context(tc.tile_pool(name="weights", bufs=1))
        sync = nc.sync

        def wload(ap, shape, tag, dt=FP16, nchunk=1):
            t = wpool.tile(list(shape), dt, tag=tag)
            if nchunk == 1:
                sync.dma_start(t[:], ap)
            else:
                step = shape[1] // nchunk
                for c in range(nchunk):
                    sl = slice(c * step, (c + 1) * step)
                    sync.dma_start(t[:, sl], ap[:, sl])
            return t

        xN = kb.act.tile([C4, BPC, T], F32, tag="nat")
        x_r = x_d.rearrange("b p t -> p b t")
        for b in range(BPC):
            sync.dma_start(xN[:, b, :], x_r[:, b, :])
        W_k1 = wload(w_k1, (128, 9, 4, 4, 128), "wk1", nchunk=3)
        W_tm1k1 = wload(w_tm1k1, (128, 9, 4, 128), "wtm1k1")
        W_tm1k2 = wload(w_tm1k2, (128, 5, 4, 128), "wtm1k2")
        W_g1 = wload(w_g1, (128, 4, 4, 128), "wg1")
        B_g1 = wload(b_g1, (128, 4, 3), "bg1", F32)
        W_k2 = wload(w_k2, (128, 9, 4, 4, 128), "wk2", nchunk=3)
        W_tm2k1 = wload(w_tm2k1, (128, 9, 4, 128), "wtm2k1")
        W_tm2k2 = wload(w_tm2k2, (128, 5, 4, 128), "wtm2k2")
        W_g2 = wload(w_g2, (128, 4, 4, 128), "wg2")
        B_g2 = wload(b_g2, (128, 4, 3), "bg2", F32)
        LN1W = wload(ln1w_d, (128, 4, NTOK), "ln1w", F32) if use_lnw1 else None
        LN1B = wload(ln1b_d, (128, 4, NTOK), "ln1b", F32) if use_lnb1 else None
        LN2W = wload(ln2w_d, (128, 4, NTOK), "ln2w", F32) if use_lnw2 else None
        LN2B = wload(ln2b_d, (128, 4, NTOK), "ln2b", F32) if use_lnb2 else None

        stats1 = kb.tiny.tile([128, 10], F32, name="stats1")
        u_x = kb.act.tile([128, 4, NTOK], F32, tag="ux")
        for b in range(BPC):
            for k in range(4):
                pt = kb.psum.tile([128, 128], F32, tag="ptr")
                nc.tensor.transpose(pt[:], xN[:, b, ts(k, 128)], kb.ident[:])
                nc.scalar.activation(
                    u_x[:, k, ts(b, 128)], pt[:], AF.Identity,
                    bias=kb.cst[:, 8:9], scale=2.5,
                    accum_out=stats1[:, 4 * b + k : 4 * b + k + 1],
                )
            sqx = kb.sqscr.tile([128, T], F32, tag="sqx", name=f"sqx{b}")
            nc.scalar.activation(
                sqx[:], xN[:, b, :], AF.Square,
                accum_out=stats1[:, 8 + b : 9 + b],
            )
        aa1, mu1 = kb.ln_tiny(stats1, [0, 0, 0, 0, 1, 1, 1, 1], True, "ln1")

        _h = {}

        def _mid1():
            a1, d1 = kb.ln_rsqrt(aa1, mu1, True, "ln1")
            _h["z1"] = kb.ln_apply(u_x, a1, d1, "z1o", LN1W, LN1B)

        BX, joX = kb.feat_big(u_x, "x", ndve=4, mid_cb=_mid1)
        z1 = _h["z1"]
        BA, joA = kb.feat_big(z1, "a", ndve=0)

        stats2 = kb.tiny.tile([128, 10], F32, name="stats2")
        cm_u = kb.act.tile([128, 4, NTOK], F32, tag="cmy")

        def cm_cb(m, pm):
            for b in range(BPC):
                nc.scalar.activation(
                    cm_u[:, m, ts(b, 128)], pm[:, ts(b, 128)], AF.Identity,
                    bias=kb.cst[:, 8:9], scale=2.5,
                    accum_out=stats2[:, 4 * b + m : 4 * b + m + 1],
                )

        kb.kan512(BX, W_k1, joX, cm_cb)
        for b in range(BPC):
            sqc = kb.sqscr.tile([128, 4, 128], F32, tag="sqc", name=f"sqc{b}")
            nc.scalar.activation(
                sqc[:], cm_u[:, :, ts(b, C4)], AF.Square,
                accum_out=stats2[:, 8 + b : 9 + b],
            )
        aa2, mu2 = kb.ln_tiny(stats2, [0, 0, 0, 0, 1, 1, 1, 1], False, "ln2")
        a2, d2 = kb.ln_rsqrt(aa2, mu2, False, "ln2")
        z3 = kb.ln_apply(cm_u, a2, d2, "z3u", LN2W, LN2B, eng=nc.vector)

        BC, joC = kb.feat_big(z3, "c", ndve=4)

        p1 = kb.kan512_to_64dup(BA, W_tm1k1, joA, "p1")
        z2 = kb.act.tile([128, NTOK], F32, tag="z2u", name="z2u")
        nc.vector.tensor_scalar(
            out=z2[:], in0=p1[:], scalar1=2.5, scalar2=5.5,
            op0=AL.mult, op1=AL.add,
        )

        p2 = kb.kan512_to_64dup(BC, W_tm2k1, joC, "p2")
        z4 = kb.act.tile([128, NTOK], F32, tag="z4u", name="z4u")
        nc.vector.tensor_scalar(
            out=z4[:], in0=p2[:], scalar1=2.5, scalar2=5.5,
            op0=AL.mult, op1=AL.add,
        )
        BD = kb.feat_small(z4, "d")
        BB = kb.feat_small(z2, "b")

        tm2 = kb.sig.tile([128, 4, NTOK], FP16, tag="tmu", name="tm2")
        kb.kan64_to_512(
            BD, W_tm2k2,
            lambda m, pm: nc.vector.tensor_add(tm2[:, m, :], cm_u[:, m, :],
                                               pm[:]),
        )
        y2u = kb.gcn(tm2, W_g2, B_g2, "y2u", u_out=True)

        tm1 = kb.sig.tile([128, 4, NTOK], FP16, tag="tmu", name="tm1")
        kb.kan64_to_512(
            BB, W_tm1k2,
            lambda m, pm: nc.vector.tensor_add(tm1[:, m, :], u_x[:, m, :],
                                               pm[:]),
        )
        y1 = kb.gcn(tm1, W_g1, B_g1, "cmy", u_out=False)

        BY, joY = kb.feat_big(y2u, "y", ndve=4)
        outT = kb.act.tile([128, 4, NTOK], F32, tag="z1o", name="outT")
        kb.kan512(
            BY, W_k2, joY,
            lambda m, pm: nc.vector.tensor_add(outT[:, m, :], y1[:, m, :],
                                               pm[:]),
        )

        outN = kb.act.tile([C4, BPC, T], F32, tag="nat", name="outN")
        out_r = out_d.rearrange("b p t -> p b t")
        for m in range(4):
            for b in range(BPC):
                pt = kb.psum.tile([128, 128], F32, tag="ptr")
                nc.tensor.transpose(pt[:], outT[:, m, ts(b, 128)], kb.ident[:])
                nc.scalar.copy(outN[:, b, ts(m, 128)], pt[:])
        for b in range(BPC):
            sync.dma_start(out_r[:, b, :], outN[:, b, :])

    return dram


def _build(ln_flags):
    key = ln_flags
    if key in _COMPILED:
        return _COMPILED[key]
    nc = bacc.Bacc("TRN2", target_bir_lowering=False, debug=False)
    _emit(nc, ln_flags)
    nc.compile()
    _COMPILED[key] = nc
    return nc


def _prep_kan512(base_w, spline_w):
    w = np.empty((128, 9, 4, 4, 128), np.float32)
    for k in range(4):
        for m in range(4):
            w[:, 0, k, m, :] = base_w[m * 128 : (m + 1) * 128,
                                      k * 128 : (k + 1) * 128].T
            for g in range(8):
                w[:, 1 + g, k, m, :] = (
                    spline_w[m * 128 : (m + 1) * 128,
                             k * 128 : (k + 1) * 128, g].T / 6.0
                )
    return np.ascontiguousarray(w.astype(np.float16))


def _prep_kan512_to_64dup(base_w, spline_w):
    w = np.empty((128, 9, 4, 128), np.float32)
    for k in range(4):
        blk = base_w[:, k * 128 : (k + 1) * 128].T
        w[:, 0, k, 0:64] = blk
        w[:, 0, k, 64:128] = blk
        for g in range(8):
            sblk = spline_w[:, k * 128 : (k + 1) * 128, g].T / 6.0
            w[:, 1 + g, k, 0:64] = sblk
            w[:, 1 + g, k, 64:128] = sblk
    return np.ascontiguousarray(w.astype(np.float16))


def _prep_kan64_to_512(base_w, spline_w, scale):
    w = np.zeros((128, 5, 4, 128), np.float32)
    for m in range(4):
        w[0:64, 0, m, :] = base_w[m * 128 : (m + 1) * 128, :].T * scale
        for i in range(4):
            w[0:64, 1 + i, m, :] = (
                spline_w[m * 128 : (m + 1) * 128, :, i].T * (scale / 6.0)
            )
            w[64:128, 1 + i, m, :] = (
                spline_w[m * 128 : (m + 1) * 128, :, i + 4].T * (scale / 6.0)
            )
    return np.ascontiguousarray(w.astype(np.float16))


def _prep_gcn(gw, gb):
    Wf = gw[:, :512] + gw[:, 512:1024] + gw[:, 1024:]
    w = np.empty((128, 4, 4, 128), np.float32)
    for k in range(4):
        for m in range(4):
            w[:, k, m, :] = (
                Wf[m * 128 : (m + 1) * 128, k * 128 : (k + 1) * 128].T / 2.5
            )
    badj = gb - 2.2 * Wf.sum(axis=1)
    b = np.empty((128, 4, 3), np.float32)
    b[:, :, 0] = badj.reshape(4, 128).T
    b[:, :, 1] = b[:, :, 0] * ISQ2
    b[:, :, 2] = b[:, :, 0] * 0.5
    return np.ascontiguousarray(w.astype(np.float16)), np.ascontiguousarray(b)


def _ln_planes(w, b):
    W = np.empty((128, 4, NTOK), np.float32)
    Bp = np.empty((128, 4, NTOK), np.float32)
    bb = 2.5 * b + 5.5 * (1.0 - w)
    for k in range(4):
        for bt in range(BPC):
            W[:, k, bt * C4 : (bt + 1) * C4] = w[k * 128 : (k + 1) * 128, :]
            Bp[:, k, bt * C4 : (bt + 1) * C4] = bb[k * 128 : (k + 1) * 128, :]
    return np.ascontiguousarray(W), np.ascontiguousarray(Bp)


def kernel(**inputs):
    i = {k: np.asarray(v) for k, v in inputs.items()}
    use_lnw1 = not np.all(i["tm1_ln_w"] == 1.0)
    use_lnb1 = not np.all(i["tm1_ln_b"] == 0.0)
    use_lnw2 = not np.all(i["tm_ln_w"] == 1.0)
    use_lnb2 = not np.all(i["tm_ln_b"] == 0.0)
    ln_flags = (use_lnw1, use_lnb1, use_lnw2, use_lnb2)
    nc = _build(ln_flags)

    w_g1, b_g1 = _prep_gcn(i["g1_w"], i["g1_b"])
    w_g2, b_g2 = _prep_gcn(i["g2_w"], i["g2_b"])
    shared = dict(
        w_k1=_prep_kan512(i["k1_base"], i["k1_spline"]),
        w_tm1k1=_prep_kan512_to_64dup(i["tm1_k1_base"], i["tm1_k1_spline"]),
        w_tm1k2=_prep_kan64_to_512(i["tm1_k2_base"], i["tm1_k2_spline"], 2.5),
        w_g1=w_g1, b_g1=b_g1,
        w_tm2k1=_prep_kan512_to_64dup(i["tm_k1_base"], i["tm_k1_spline"]),
        w_tm2k2=_prep_kan64_to_512(i["tm_k2_base"], i["tm_k2_spline"], 2.5),
        w_g2=w_g2, b_g2=b_g2,
        w_k2=_prep_kan512(i["k2_base"], i["k2_spline"]),
    )
    if use_lnw1 or use_lnb1:
        W, Bp = _ln_planes(i["tm1_ln_w"], i["tm1_ln_b"])
        if use_lnw1:
            shared["ln1w"] = W
        if use_lnb1:
            shared["ln1b"] = Bp
    if use_lnw2 or use_lnb2:
        W, Bp = _ln_planes(i["tm_ln_w"], i["tm_ln_b"])
        if use_lnw2:
            shared["ln2w"] = W
        if use_lnb2:
            shared["ln2b"] = Bp
    x = np.ascontiguousarray(i["x"], np.float32)
    in_maps = [
        {"x_sh": x[c * BPC : (c + 1) * BPC], **shared} for c in range(NCORES)
    ]
    res = run_bass_kernel_spmd(nc, in_maps, core_ids=list(range(NCORES)))
    out = np.empty((B, C4, T), np.float32)
    for c in range(NCORES):
        out[c * BPC : (c + 1) * BPC] = res.results[c]["out_sh"]
    return out
